# revision 8
# baseline (speedup 1.0000x reference)
"""RGCN (3-layer, basis-regularized) on 8 Trainium2 NeuronCores.

Strategy (per spec sharding hint): shard nodes across the 8 cores; partition
edges by destination so each core owns its segment-mean reductions; gather
source features from replicated bf16 DRAM tables via dma_gather; segment-sum
via one-hot-weighted bf16 matmuls on TensorE accumulating directly into a
PSUM-resident per-sub-range accumulator at per-group dynamic offsets
(inv-degree folded into the one-hot weights on host); per-relation weights
(basis-combined on host) applied with fp32 matmuls; AllGather of each core's
node shard between layers.

Self-contained: hardcodes all shapes from the problem spec.
"""
import sys

sys.path.insert(0, "/opt/trn_rl_repo")

import numpy as np

# ---------------- problem constants (hardcoded per spec) ----------------
R = 8          # relations
N = 50000      # nodes
E = 400000     # edges per relation
D = 128        # hidden dim
C = 16         # classes
B = 4          # bases
P = 8          # cores
NS = N // P    # 6250 nodes per core

HALF = N // 2      # src-half split (int16 gather index limit)
SUBW = NS // 2     # 3125: dst sub-range width (PSUM accumulator capacity)
NSTREAM = R * 2 * 2            # 32 streams per core: (r, half, sub)
GPS = 102                      # groups (128 edges) per stream
LS = GPS * 128                 # 13056 padded edges per stream
CHUNK_G = 51                   # groups per gather chunk
CHUNK_I = CHUNK_G * 128        # 6528 indices per gather
NCHUNK = GPS // CHUNK_G        # 2
LOADB = 17  # overridden below via K_LOADB
PSW = 3328                     # PSUM accumulator width (>= SUBW-1+128, 7 banks)
ACCW = 6656                    # SBUF accumulator width (13 x 512)
TT = 49                        # 128-col transpose tiles covering 6250 rows

_BUILT = None
import os
NLAYERS = int(os.environ.get('K_LAYERS', '3'))
USE_CC = os.environ.get('K_CC', '1') == '1'
LOADB_ENV = int(os.environ.get('K_LOADB', '0'))
MEMSET_DVE = os.environ.get('K_MSDVE', '0') == '1'


def _build():
    from concourse import bass, bacc, tile, mybir

    f32 = mybir.dt.float32
    bf16 = mybir.dt.bfloat16
    i16 = mybir.dt.int16
    i32 = mybir.dt.int32
    PE = mybir.EngineType.PE

    nc = bacc.Bacc("TRN2", target_bir_lowering=False, debug=False, num_devices=P)

    emb = nc.dram_tensor("emb", [R, N, D], bf16, kind="ExternalInput")
    idx_d = nc.dram_tensor("idxw", [NSTREAM, 128, LS // 16], i16, kind="ExternalInput")
    dpp_d = nc.dram_tensor("dpp", [128, NSTREAM * GPS], f32, kind="ExternalInput")
    wgt_d = nc.dram_tensor("wgt", [128, NSTREAM * GPS], f32, kind="ExternalInput")
    wb_d = nc.dram_tensor("wb", [1, NSTREAM * GPS], i32, kind="ExternalInput")
    iota_d = nc.dram_tensor("iota", [128, 128], f32, kind="ExternalInput")
    ident_d = nc.dram_tensor("ident", [128, 128], bf16, kind="ExternalInput")
    w1_d = nc.dram_tensor("w1", [D, R * D], f32, kind="ExternalInput")
    w2_d = nc.dram_tensor("w2", [D, R * C], f32, kind="ExternalInput")
    eb_d = nc.dram_tensor("eb", [D, 1], f32, kind="ExternalInput")
    b1_d = nc.dram_tensor("b1", [D, 1], f32, kind="ExternalInput")
    b2_d = nc.dram_tensor("b2", [C, 1], f32, kind="ExternalInput")
    out_d = nc.dram_tensor("out", [C, NS], f32, kind="ExternalOutput")

    with tile.TileContext(nc) as tc:
        with tc.tile_pool(name="const", bufs=1) as cp, tc.tile_pool(
            name="idxp", bufs=2
        ) as idxp, tc.tile_pool(name="msgp", bufs=3) as msgp, tc.tile_pool(
            name="sp", bufs=6
        ) as sp, tc.tile_pool(name="accp", bufs=1, space="PSUM") as accp, tc.tile_pool(
            name="smallp", bufs=1, space="PSUM"
        ) as smallp, tc.tile_pool(name="dram", bufs=1, space="DRAM") as dp:
            iota_t = cp.tile([128, 128], f32)
            nc.sync.dma_start(iota_t[:], iota_d.ap())
            ident_t = cp.tile([128, 128], bf16)
            nc.sync.dma_start(ident_t[:], ident_d.ap())
            dpp_t = cp.tile([128, NSTREAM * GPS], f32)
            nc.sync.dma_start(dpp_t[:], dpp_d.ap())
            wgt_t = cp.tile([128, NSTREAM * GPS], f32)
            nc.sync.dma_start(wgt_t[:], wgt_d.ap())
            wb_t = cp.tile([1, NSTREAM * GPS], i32)
            nc.sync.dma_start(wb_t[:], wb_d.ap())
            w1_t = cp.tile([D, R * D], f32)
            nc.sync.dma_start(w1_t[:], w1_d.ap())
            w2_t = cp.tile([D, R * C], f32)
            nc.sync.dma_start(w2_t[:], w2_d.ap())
            eb_t = cp.tile([D, 1], f32)
            nc.sync.dma_start(eb_t[:], eb_d.ap())
            b1_t = cp.tile([D, 1], f32)
            nc.sync.dma_start(b1_t[:], b1_d.ap())
            b2_t = cp.tile([C, 1], f32)
            nc.sync.dma_start(b2_t[:], b2_d.ap())

            acc = cp.tile([128, ACCW], f32)
            nc.vector.memset(acc[:], 0.0)  # cols >= 6250 stay zero forever
            outacc = cp.tile([128, ACCW], f32)
            hbf = cp.tile([128, TT * 128], bf16)
            hT = cp.tile([128, TT, 128], bf16)
            outf = cp.tile([C, NS], f32)

            acc_ps = accp.tile([128, PSW], f32)

            hbounce = dp.tile([TT * 128, D], bf16)
            hfull = [
                dp.tile([N, D], bf16, addr_space="Shared", name=f"hfull{i}")
                for i in range(2)
            ]

            def agg_stream(layer, r, h, sub):
                """Gather + one-hot matmuls of one (r, half, sub) stream into acc_ps."""
                sid = (r * 2 + h) * 2 + sub
                if layer == 0:
                    tv = emb.ap()[r, h * HALF : (h + 1) * HALF, :]
                else:
                    tv = hfull[layer - 1][h * HALF : (h + 1) * HALF, :]
                idx_sb = idxp.tile([128, LS // 16], i16, tag="idx")
                nc.sync.dma_start(idx_sb[:], idx_d.ap()[sid, :, :])
                for ch in range(NCHUNK):
                    msgs = msgp.tile([128, CHUNK_G, 128], bf16, tag="msgs")
                    nc.gpsimd.dma_gather(
                        out_ap=msgs[:],
                        in_ap=tv,
                        idxs_ap=idx_sb[
                            :, ch * (CHUNK_I // 16) : (ch + 1) * (CHUNK_I // 16)
                        ],
                        num_idxs=CHUNK_I,
                        num_idxs_reg=CHUNK_I,
                        elem_size=D,
                        single_packet=False,
                    )
                    _lb = LOADB_ENV or LOADB
                    for lb in range(CHUNK_G // _lb):
                        g0 = sid * GPS + ch * CHUNK_G + lb * _lb
                        _, bases = nc.values_load_multi_w_load_instructions(
                            wb_t[0:1, g0 : g0 + _lb],
                            engines=[PE],
                            min_val=0,
                            max_val=PSW - 128,
                            skip_runtime_bounds_check=True,
                        )
                        for k in range(_lb):
                            lg = lb * _lb + k
                            col = sid * GPS + ch * CHUNK_G + lg
                            s_t = sp.tile([128, 128], bf16, tag="s")
                            nc.vector.tensor_scalar(
                                s_t[:],
                                iota_t[:],
                                dpp_t[:, col : col + 1],
                                wgt_t[:, col : col + 1],
                                mybir.AluOpType.is_equal,
                                mybir.AluOpType.mult,
                            )
                            nc.tensor.matmul(
                                out=acc_ps[:, bass.ds(bases[k], 128)],
                                lhsT=msgs[:, lg, :],
                                rhs=s_t[:],
                                start=False,
                                stop=True,
                            )

            for layer in range(NLAYERS):
                if layer == 0:
                    for sub in range(2):
                        (nc.vector if MEMSET_DVE else nc.any).memset(acc_ps[:], 0.0)
                        for r in range(R):
                            for h in range(2):
                                agg_stream(layer, r, h, sub)
                        # bias + relu + cast directly from PSUM
                        nc.vector.tensor_scalar(
                            hbf[:, sub * SUBW : sub * SUBW + SUBW],
                            acc_ps[:, 0:SUBW],
                            eb_t[:],
                            0.0,
                            mybir.AluOpType.add,
                            mybir.AluOpType.max,
                        )
                else:
                    nc.any.memset(outacc[:], 0.0)
                    M = D if layer == 1 else C
                    w_t = w1_t if layer == 1 else w2_t
                    for r in range(R):
                        for sub in range(2):
                            (nc.vector if MEMSET_DVE else nc.any).memset(acc_ps[:], 0.0)
                            for h in range(2):
                                agg_stream(layer, r, h, sub)
                            nc.scalar.copy(
                                acc[:, sub * SUBW : sub * SUBW + SUBW],
                                acc_ps[:, 0:SUBW],
                            )
                        for ch2 in range(ACCW // 512):
                            p2 = smallp.tile([128, 512], f32, tag="small", name="p2")
                            nc.tensor.matmul(
                                out=p2[0:M, :],
                                lhsT=w_t[:, r * M : (r + 1) * M],
                                rhs=acc[:, ch2 * 512 : (ch2 + 1) * 512],
                                start=True,
                                stop=True,
                            )
                            nc.vector.tensor_tensor(
                                out=outacc[0:M, ch2 * 512 : (ch2 + 1) * 512],
                                in0=outacc[0:M, ch2 * 512 : (ch2 + 1) * 512],
                                in1=p2[0:M, :],
                                op=mybir.AluOpType.add,
                            )

                if layer < 2 and NLAYERS == 3:
                    if layer == 1:
                        # bias + relu + cast from SBUF outacc
                        nc.vector.tensor_scalar(
                            hbf[:],
                            outacc[:, 0 : TT * 128],
                            b1_t[:],
                            0.0,
                            mybir.AluOpType.add,
                            mybir.AluOpType.max,
                        )
                    for t in range(TT):
                        pt = smallp.tile([128, 128], bf16, tag="small", name="pt")
                        nc.tensor.transpose(
                            pt[:], hbf[:, t * 128 : (t + 1) * 128], ident_t[:]
                        )
                        nc.scalar.copy(hT[:, t, :], pt[:])
                    nc.sync.dma_start(
                        hbounce[:].rearrange("(w p) d -> p w d", p=128), hT[:]
                    )
                    if not USE_CC:
                        nc.gpsimd.dma_start(hfull[layer][0:NS, :], hbounce[0:NS, :])
                    else:
                        nc.gpsimd.collective_compute(
                            "AllGather",
                            mybir.AluOpType.bypass,
                            replica_groups=[list(range(P))],
                            ins=[hbounce[0:NS, :].opt()],
                            outs=[hfull[layer][:].opt()],
                        )
                elif layer == 2:
                    nc.vector.tensor_scalar_add(outf[:], outacc[0:C, 0:NS], b2_t[:])
                    nc.sync.dma_start(out_d.ap(), outf[:])

    if NLAYERS < 3:
        pass
    nc.compile()
    return nc


def _preprocess(edge_src, edge_dst, embeds, w1_basis, w1_comp, w2_basis, w2_comp):
    """Host-side sharding: edge partitioning, sorting, grouping, weights."""
    import ml_dtypes

    bf16 = ml_dtypes.bfloat16

    edge_src = np.asarray(edge_src, dtype=np.int64)
    edge_dst = np.asarray(edge_dst, dtype=np.int64)

    idx_all = np.zeros((P, NSTREAM, 128, LS // 16), dtype=np.int16)
    dpp = np.full((P, 128, NSTREAM * GPS), -1.0, dtype=np.float32)
    wgt = np.zeros((P, 128, NSTREAM * GPS), dtype=np.float32)
    wb = np.zeros((P, 1, NSTREAM * GPS), dtype=np.int32)

    for r in range(R):
        src = edge_src[r]
        dst = edge_dst[r]
        deg = np.bincount(dst, minlength=N).astype(np.float32)
        inv = (1.0 / np.maximum(deg, 1.0)).astype(np.float32)
        core = dst // NS
        half = src // HALF
        dloc = dst - core * NS
        sub = dloc // SUBW
        key = (core * 2 + half) * 2 + sub
        order = np.lexsort((dst, key))
        s_key, s_src, s_dst, s_dloc = (
            key[order],
            src[order],
            dst[order],
            dloc[order],
        )
        bounds = np.searchsorted(s_key, np.arange(4 * P + 1))
        for c in range(P):
            for h in range(2):
                for sb in range(2):
                    k4 = (c * 2 + h) * 2 + sb
                    lo, hi = bounds[k4], bounds[k4 + 1]
                    cnt = hi - lo
                    assert cnt <= LS, f"stream overflow {cnt} > {LS}"
                    e_src = (s_src[lo:hi] - h * HALF).astype(np.int64)
                    e_ds = (s_dloc[lo:hi] - sb * SUBW).astype(np.int64)
                    e_w = inv[s_dst[lo:hi]]

                    idxs = np.zeros(LS, dtype=np.int16)
                    idxs[:cnt] = e_src
                    d2 = np.full(LS, -1.0, dtype=np.float32)
                    ww = np.zeros(LS, dtype=np.float32)
                    bases = np.zeros(GPS, dtype=np.int32)
                    ng = (cnt + 127) // 128
                    for g in range(ng):
                        a = g * 128
                        b = min(cnt, a + 128)
                        base = int(e_ds[a])
                        width = int(e_ds[b - 1]) - base
                        assert 0 <= width <= 127, f"group width {width}"
                        bases[g] = base
                        d2[a:b] = e_ds[a:b] - base
                        ww[a:b] = e_w[a:b]

                    sid = (r * 2 + h) * 2 + sb
                    idx_all[c, sid] = np.tile(idxs.reshape(-1, 16).T, (8, 1))
                    dpp[c, :, sid * GPS : (sid + 1) * GPS] = d2.reshape(GPS, 128).T
                    wgt[c, :, sid * GPS : (sid + 1) * GPS] = ww.reshape(GPS, 128).T
                    wb[c, 0, sid * GPS : (sid + 1) * GPS] = bases

    emb_bf = np.ascontiguousarray(embeds).astype(bf16)
    W1c = np.einsum("rb,bio->rio", w1_comp, w1_basis).astype(np.float32)
    W2c = np.einsum("rb,bio->rio", w2_comp, w2_basis).astype(np.float32)
    w1_p = np.ascontiguousarray(np.transpose(W1c, (1, 0, 2)).reshape(D, R * D))
    w2_p = np.ascontiguousarray(np.transpose(W2c, (1, 0, 2)).reshape(D, R * C))
    iota_mat = np.tile(np.arange(128, dtype=np.float32), (128, 1))
    ident = np.eye(128, dtype=np.float32).astype(bf16)
    return idx_all, dpp, wgt, wb, emb_bf, w1_p, w2_p, iota_mat, ident


def kernel(
    edge_src,
    edge_dst,
    embeds,
    emb_bias,
    w1_basis,
    w1_comp,
    b1,
    w2_basis,
    w2_comp,
    b2,
):
    global _BUILT
    from concourse import bass_utils

    if _BUILT is None:
        _BUILT = _build()
    nc = _BUILT

    idx_all, dpp, wgt, wb, emb_bf, w1_p, w2_p, iota_mat, ident = _preprocess(
        edge_src, edge_dst, embeds, w1_basis, w1_comp, w2_basis, w2_comp
    )

    shared = {
        "emb": emb_bf,
        "iota": iota_mat,
        "ident": ident,
        "w1": w1_p,
        "w2": w2_p,
        "eb": np.asarray(emb_bias, np.float32).reshape(D, 1),
        "b1": np.asarray(b1, np.float32).reshape(D, 1),
        "b2": np.asarray(b2, np.float32).reshape(C, 1),
    }
    in_maps = [
        {
            **shared,
            "idxw": idx_all[c],
            "dpp": dpp[c],
            "wgt": wgt[c],
            "wb": wb[c],
        }
        for c in range(P)
    ]
    res = bass_utils.run_bass_kernel_spmd(nc, in_maps, core_ids=list(range(P)))
    out = np.concatenate(
        [res.results[c]["out"].T for c in range(P)], axis=0
    ).astype(np.float32)
    return out
